# revision 2
# baseline (speedup 1.0000x reference)
"""Trainium2 Bass kernel for DiffMultiHeadedAttention (differential attention).

Model (per reference):
    q = x @ Wq.T + bq; k = ef @ Wk.T + bk; v = ef @ Wv.T + bv
    lambda_full = exp(sum(lq1*lk1)) - exp(sum(lq2*lk2)) + 0.8
    att  = softmax(causal_mask(q_hh @ k_hh.T / sqrt(32)))   per 32 half-heads
    out_h = (att[2h] - lambda_full * att[2h+1]) @ v_h       per 16 heads
B=4, T=N=1024, H=16 heads of 64, 2H=32 half-heads of 32.

Sharding over 8 cores: core c = (batch b = c//2, head-group hg = c%2).
Each core owns one batch element and 8 full heads (16 half-heads) and
computes out[t, o] [1024, 512]; the host reassembles (no transpose).

Design (v2, restructured for PE/Scalar overlap):
  - qk: per (oc, tcv, nt) the FOUR half-head matmuls (K=32) go to the four
    32-row PE strips via tile_position=(base, 0) -> concurrent execution,
    one 4-bank PSUM tile [128, 4, 512].
  - exp: one Scalar ACTIVATE per (oc, tcv, nt) over [128, 4, w] (the Scalar
    engine is the phase-2 floor at ~(N+352)/1.2 ns; bigger tiles amortize
    the 352-cycle fixed cost). Triangular mask multiplied post-exp on DVE
    for diagonal tiles only.
  - av TRANSPOSED: E tiles are the stationary operand, vaug [128n, 65]
    ([v_h | 1]) streams -> PSUM chains [128t, 65] per (head, s, t-chunk),
    accumulated over n-tiles. Output lands t-on-partitions, so the softmax
    denominator (col 64) is a per-partition scalar: the diff-softmax
    combine is plain DVE tensor_scalar work, no gpsimd partition-broadcast
    and no accumulating DMA. Output DMA'd per (t-chunk, oc) as [128, 128].
  - Projections (q/k per-oc, v per n-tile) are chopped into chain units and
    interleaved between qk slots as PE filler while exp paces the sweep;
    DMAs are issued in dependency priority order (efT, wk[oc0], wvT, xT,
    wq[oc0], rest) and ~4.5us of dummy matmuls warm the PE HAM clock-gate
    during the initial DMA fill.
"""

import math

import numpy as np

B, T, N, HIDDEN = 4, 1024, 1024, 1024
H, HEAD, HALF = 16, 64, 32
O = 512            # per-core hidden slice (8 heads * 64)
HPC = 8            # heads per core
LAMBDA_INIT = 0.8
SCALE = 1.0 / math.sqrt(HALF)
P = 128
IC = HIDDEN // P   # 8 contraction chunks
OC = O // P        # 4 output chunks of the projections
NT = N // P        # 8 n-tiles (keys)
NCORES = 8

_STATE = {}


def _build_nc():
    from contextlib import ExitStack

    import concourse.bacc as bacc
    import concourse.mybir as mybir
    import concourse.tile as tile
    from concourse.bass import ts

    f32 = mybir.dt.float32
    f16 = mybir.dt.float16
    AF = mybir.ActivationFunctionType
    ALU = mybir.AluOpType

    nc = bacc.Bacc("TRN2", target_bir_lowering=False, debug=False)

    xt_d = nc.dram_tensor("xt", [HIDDEN, T], f16, kind="ExternalInput")
    eft_d = nc.dram_tensor("eft", [HIDDEN, N], f16, kind="ExternalInput")
    wqt_d = nc.dram_tensor("wqt", [HIDDEN, O], f16, kind="ExternalInput")
    wkt_d = nc.dram_tensor("wkt", [HIDDEN, O], f16, kind="ExternalInput")
    wvt_d = nc.dram_tensor("wvt", [HIDDEN, O], f16, kind="ExternalInput")
    bq_d = nc.dram_tensor("bq", [1, O], f32, kind="ExternalInput")
    bk_d = nc.dram_tensor("bk", [1, O], f32, kind="ExternalInput")
    bv_d = nc.dram_tensor("bv", [1, O], f32, kind="ExternalInput")
    lq1_d = nc.dram_tensor("lq1", [1, HALF], f32, kind="ExternalInput")
    lq2_d = nc.dram_tensor("lq2", [1, HALF], f32, kind="ExternalInput")
    lk1_d = nc.dram_tensor("lk1", [1, HALF], f32, kind="ExternalInput")
    lk2_d = nc.dram_tensor("lk2", [1, HALF], f32, kind="ExternalInput")
    out_d = nc.dram_tensor("out", [T, O], f32, kind="ExternalOutput")

    with tile.TileContext(nc) as tc:
        with ExitStack() as ctx:
            const = ctx.enter_context(tc.tile_pool(name="const", bufs=1))
            big = ctx.enter_context(tc.tile_pool(name="big", bufs=1))

            # ---- input staging (persistent; proj interleaves into phase B) ----
            efT = big.tile([P, IC, N], f16)
            wkT = big.tile([P, IC, O], f16)
            wvT = big.tile([P, IC, O], f16)
            xT = big.tile([P, IC, T], f16)
            wqT = big.tile([P, IC, O], f16)

            # DMA priority order: k-proj(oc0) inputs, v inputs, q-proj(oc0)
            # inputs, then remaining weight columns.
            for ic in range(IC):
                nc.sync.dma_start(efT[:, ic, :], eft_d[ts(ic, P), :])
                nc.sync.dma_start(wkT[:, ic, 0:P], wkt_d[ts(ic, P), 0:P])
            for ic in range(IC):
                nc.sync.dma_start(wvT[:, ic, :], wvt_d[ts(ic, P), :])
            for ic in range(IC):
                nc.sync.dma_start(xT[:, ic, :], xt_d[ts(ic, P), :])
                nc.sync.dma_start(wqT[:, ic, 0:P], wqt_d[ts(ic, P), 0:P])
            for ic in range(IC):
                nc.sync.dma_start(wkT[:, ic, P:O], wkt_d[ts(ic, P), P:O])
                nc.sync.dma_start(wqT[:, ic, P:O], wqt_d[ts(ic, P), P:O])

            # ---- lambda_full (tiny, computed once) ----
            lam_in = const.tile([1, 4, HALF], f32)
            nc.sync.dma_start(lam_in[:, 0, :], lq1_d[:])
            nc.sync.dma_start(lam_in[:, 1, :], lk1_d[:])
            nc.sync.dma_start(lam_in[:, 2, :], lq2_d[:])
            nc.sync.dma_start(lam_in[:, 3, :], lk2_d[:])
            lam_tmp = const.tile([1, 2, HALF], f32)
            nc.vector.tensor_mul(lam_tmp[:, 0, :], lam_in[:, 0, :], lam_in[:, 1, :])
            nc.vector.tensor_mul(lam_tmp[:, 1, :], lam_in[:, 2, :], lam_in[:, 3, :])
            lam_s = const.tile([1, 2], f32)
            nc.vector.tensor_reduce(
                lam_s, lam_tmp, axis=mybir.AxisListType.X, op=ALU.add
            )
            lam_e = const.tile([1, 2], f32)
            nc.scalar.activation(lam_e, lam_s, AF.Exp)
            # lam_neg = -(e1 - e2 + 0.8) = e2 - e1 - 0.8
            lam_neg = const.tile([1, 1], f32)
            nc.vector.tensor_sub(lam_neg, lam_e[:, 1:2], lam_e[:, 0:1])
            nc.vector.tensor_scalar_add(lam_neg, lam_neg, -LAMBDA_INIT)
            # lam4 = [1, -lam, 1, -lam] per partition: one DVE mul scales the
            # four reciprocals of a combine tile in a single op.
            lam_negb = const.tile([P, 1], f32)
            nc.gpsimd.partition_broadcast(lam_negb, lam_neg)
            lam4 = const.tile([P, 4, 1], f32)
            nc.vector.memset(lam4, 1.0)
            nc.vector.tensor_copy(lam4[:, 1, :], lam_negb)
            nc.vector.tensor_copy(lam4[:, 3, :], lam_negb)

            # 0/1 upper-triangular mask (keep t_local >= n_local), x4 so one
            # DVE mul masks all four half-heads of an oc.
            tri4 = const.tile([P, 4, P], f16)
            neg3 = const.tile([P, 1], f32)
            nc.vector.memset(neg3, -3.0)
            nc.gpsimd.memset(tri4, 1.0)
            nc.gpsimd.affine_select(
                out=tri4,
                in_=tri4,
                compare_op=ALU.is_ge,
                fill=0.0,
                base=0,
                pattern=[[0, 4], [1, P]],
                channel_multiplier=-1,
            )

            # ---- biases ----
            bq_sb = const.tile([P, OC], f32)
            nc.sync.dma_start(bq_sb, bq_d[0].rearrange("(a p) -> p a", p=P))
            bk_sb = const.tile([P, OC], f32)
            nc.sync.dma_start(bk_sb, bk_d[0].rearrange("(a p) -> p a", p=P))
            bv_1 = const.tile([1, O], f32)
            nc.sync.dma_start(bv_1, bv_d[:])
            bvb = const.tile([P, O], f32)
            nc.gpsimd.partition_broadcast(bvb, bv_1)

            # ---- persistent projection outputs ----
            qT = big.tile([P, OC, T], f16)           # [d-part, oc, t]
            kT = big.tile([P, OC, N], f16)           # [d-part, oc, n]
            vaug = big.tile([P, NT, HPC, HEAD + 1], f16)  # [n-part, nt, h, d|1]
            ones8 = const.tile([P, HPC], f32)
            nc.vector.memset(ones8, 1.0)
            for nt_ in range(NT):
                nc.vector.tensor_copy(
                    vaug[:, nt_, :, HEAD : HEAD + 1],
                    ones8[:, :].rearrange("p (a b) -> p a b", b=1),
                )

            # ---- PSUM pools: qk 4 banks + av 2 + proj 2 = 8 ----
            ps_qk = ctx.enter_context(tc.tile_pool(name="ps_qk", bufs=1, space="PSUM"))
            ps_av = ctx.enter_context(tc.tile_pool(name="ps_av", bufs=2, space="PSUM"))
            ps_pj = ctx.enter_context(tc.tile_pool(name="ps_pj", bufs=2, space="PSUM"))
            att = ctx.enter_context(tc.tile_pool(name="att", bufs=1))
            outs = ctx.enter_context(tc.tile_pool(name="outs", bufs=1))

            # ---- PE warm-up: ~4.5us of dummy matmuls during the DMA fill so
            # the HAM clock-gate reaches 8/8 before real work. No consumers.
            wz = const.tile([32, 512], f16)
            nc.vector.memset(wz, 0.0)
            for i in range(18):
                wps = ps_pj.tile([P, 512], f32, tag="pj", name="warm")
                nc.tensor.matmul(
                    wps[0:1, :], wz[:, 0:1], wz[:, :], start=True, stop=True
                )

            # ---------- projection chain machinery ----------
            # Each chain (8 ic-matmuls + a DVE drain) is split into two
            # 4-matmul units so the filler granularity is ~0.85us.
            def make_kq_chain(wT, b_sb, actT, dstT, oc, t2):
                cell = {}

                def unit_a():
                    psj = ps_pj.tile([P, 512], f32, tag="pj", name="psj")
                    cell["psj"] = psj
                    for ic in range(4):
                        nc.tensor.matmul(
                            psj,
                            wT[:, ic, ts(oc, P)],
                            actT[:, ic, ts(t2, 512)],
                            start=(ic == 0),
                            stop=False,
                        )

                def unit_b():
                    psj = cell["psj"]
                    for ic in range(4, IC):
                        nc.tensor.matmul(
                            psj,
                            wT[:, ic, ts(oc, P)],
                            actT[:, ic, ts(t2, 512)],
                            start=False,
                            stop=(ic == IC - 1),
                        )
                    nc.vector.tensor_scalar_add(
                        dstT[:, oc, ts(t2, 512)], psj, b_sb[:, oc : oc + 1]
                    )

                return [(4, unit_a), (4, unit_b)]

            def make_v_chain(nt_):
                cell = {}

                def unit_a():
                    psj = ps_pj.tile([P, 512], f32, tag="pj", name="psv")
                    cell["psj"] = psj
                    for ic in range(4):
                        nc.tensor.matmul(
                            psj,
                            efT[:, ic, ts(nt_, P)],
                            wvT[:, ic, :],
                            start=(ic == 0),
                            stop=False,
                        )

                def unit_b():
                    psj = cell["psj"]
                    for ic in range(4, IC):
                        nc.tensor.matmul(
                            psj,
                            efT[:, ic, ts(nt_, P)],
                            wvT[:, ic, :],
                            start=False,
                            stop=(ic == IC - 1),
                        )
                    nc.vector.tensor_add(
                        vaug[:, nt_, :, 0:HEAD],
                        psj[:].rearrange("p (h d) -> p h d", h=HPC),
                        bvb[:].rearrange("p (h d) -> p h d", h=HPC),
                    )

                return [(4, unit_a), (4, unit_b)]

            # Phase A (emitted now, DMA-paced): k(oc0), v(nt0..2), q(oc0).
            for t2 in range(2):
                for _, u in make_kq_chain(wkT, bk_sb, efT, kT, 0, t2):
                    u()
            for nt_ in range(3):
                for _, u in make_v_chain(nt_):
                    u()
            for t2 in range(2):
                for _, u in make_kq_chain(wqT, bq_sb, xT, qT, 0, t2):
                    u()

            # Filler queue for phase B (in rough dependency-priority order).
            fillers = []
            for nt_ in range(3, NT):
                fillers.extend(make_v_chain(nt_))
            for oc in range(1, OC):
                for t2 in range(2):
                    fillers.extend(make_kq_chain(wkT, bk_sb, efT, kT, oc, t2))
                for t2 in range(2):
                    fillers.extend(make_kq_chain(wqT, bq_sb, xT, qT, oc, t2))
            fq = list(reversed(fillers))  # pop() from the front

            # ---------- phase B: attention ----------
            for oc in range(OC):
                Es = {}
                for tcv in (1, 0):
                    nis = range(NT) if tcv == 1 else range(4)
                    for nt_ in nis:
                        t0 = nt_ * P
                        cs = max(t0, 512 * tcv)
                        w = 512 * (tcv + 1) - cs
                        # qk: four half-heads concurrently in the four
                        # 32-row PE strips, four PSUM banks.
                        qkps = ps_qk.tile([P, 4, 512], f32, tag="qk", name="qkps")
                        for j in range(2):
                            for s in range(2):
                                base = 64 * j + 32 * s
                                nc.tensor.matmul(
                                    qkps[:, 2 * j + s, :w],
                                    kT[base : base + 32, oc, ts(nt_, P)],
                                    qT[base : base + 32, oc, cs : cs + w],
                                    start=True,
                                    stop=True,
                                    tile_position=(base, 0),
                                )
                        E = att.tile([P, 4, 512], f16, tag="E", bufs=14, name="E")
                        # bias shifts all exps by e^-3 (cancels in P/s),
                        # keeping E inside fp16 range
                        nc.scalar.activation(
                            E[:, :, :w],
                            qkps[:, :, :w],
                            AF.Exp,
                            bias=neg3[:, 0:1],
                            scale=SCALE,
                        )
                        if cs == t0:
                            # diagonal block: keep t_local >= n_local
                            nc.vector.tensor_mul(E[:, :, 0:P], E[:, :, 0:P], tri4)
                        Es[(tcv, nt_)] = E

                        # av chains for t-chunk tc == nt (global), then the
                        # per-partition diff-softmax combine + output DMA.
                        tc_ = nt_
                        av_units = 0.0
                        if tcv == 0 or nt_ >= 4:
                            avp = ps_av.tile([P, 4, 65], f32, tag="av", name="avp")
                            for j in range(2):
                                h = 2 * oc + j
                                for s in range(2):
                                    for ntp in range(tc_ + 1):
                                        Ew = Es[(tcv, ntp)]
                                        csp = max(ntp * P, 512 * tcv)
                                        c0 = tc_ * P - csp
                                        nc.tensor.matmul(
                                            avp[:, 2 * j + s, :],
                                            Ew[:, 2 * j + s, c0 : c0 + P],
                                            vaug[:, ntp, h, :],
                                            start=(ntp == 0),
                                            stop=(ntp == tc_),
                                        )
                            av_units = 0.75 * 4 * (tc_ + 1) * 65 / 512
                            R = outs.tile([P, 4, 1], f32, tag="R", bufs=4, name="R")
                            nc.vector.reciprocal_approx_fast(
                                out=R, in_=avp[:, :, 64:65]
                            )
                            nc.vector.tensor_mul(R, R, lam4)
                            osb = outs.tile(
                                [P, 2, HEAD], f32, tag="o", bufs=4, name="osb"
                            )
                            for j in range(2):
                                nc.vector.tensor_scalar_mul(
                                    osb[:, j, :],
                                    avp[:, 2 * j + 1, 0:HEAD],
                                    R[:, 2 * j + 1, 0:1],
                                )
                                nc.vector.scalar_tensor_tensor(
                                    out=osb[:, j, :],
                                    in0=avp[:, 2 * j, 0:HEAD],
                                    scalar=R[:, 2 * j, 0:1],
                                    in1=osb[:, j, :],
                                    op0=ALU.mult,
                                    op1=ALU.add,
                                )
                            nc.sync.dma_start(
                                out_d[ts(tc_, P), ts(oc, P)],
                                osb[:].rearrange("p a b -> p (a b)"),
                            )

                        # greedy filler: keep the PE fed while exp paces
                        window = (4 * w + 352) / 1.2 / 213.0
                        acc = w / 512.0 + av_units
                        while fq and acc < window - 1.0:
                            cost, u = fq.pop()
                            u()
                            acc += cost * 0.27
            # drain any leftover projection work (shouldn't happen)
            while fq:
                _, u = fq.pop()
                u()

    nc.compile()
    return nc


def _ensure_axon_hooks():
    """concourse's trace path imports antenv.axon_hooks, which this image
    lacks; provide it (registering the real ctypes NTFF hook when available)
    so BASS_TRACE=1 degrades gracefully instead of crashing."""
    import sys
    import types

    if "antenv.axon_hooks" in sys.modules:
        return
    try:
        import antenv.axon_hooks  # noqa: F401

        return
    except ImportError:
        pass
    mod = types.ModuleType("antenv.axon_hooks")
    mod._hook = None
    mod.set_axon_ntff_profile_hook = lambda h: setattr(mod, "_hook", h)
    mod.get_axon_ntff_profile_hook = lambda: mod._hook
    sys.modules["antenv.axon_hooks"] = mod
    import os

    if os.environ.get("KERNEL_TRACE") == "1":
        try:
            from trn_agent_boot.trn_boot import _ntff_profile_via_ctypes

            mod._hook = _ntff_profile_via_ctypes("/opt/axon/libaxon_pjrt.so")
        except Exception:
            pass


def _get_state():
    if "nc" not in _STATE:
        from concourse.bass_utils import run_bass_kernel_spmd

        _ensure_axon_hooks()
        _STATE["nc"] = _build_nc()
        _STATE["run"] = run_bass_kernel_spmd
    return _STATE


def kernel(**inputs):
    st = _get_state()

    def f32c(a):
        return np.ascontiguousarray(np.asarray(a, dtype=np.float32))

    x = np.asarray(inputs["x"], dtype=np.float32)
    ef = np.asarray(inputs["encoder_feature"], dtype=np.float32)
    Wq, bq = np.asarray(inputs["Wq"], np.float32), np.asarray(inputs["bq"], np.float32)
    Wk, bk = np.asarray(inputs["Wk"], np.float32), np.asarray(inputs["bk"], np.float32)
    Wv, bv = np.asarray(inputs["Wv"], np.float32), np.asarray(inputs["bv"], np.float32)
    lq1 = f32c(inputs["lambda_q1"]).reshape(1, HALF)
    lq2 = f32c(inputs["lambda_q2"]).reshape(1, HALF)
    lk1 = f32c(inputs["lambda_k1"]).reshape(1, HALF)
    lk2 = f32c(inputs["lambda_k2"]).reshape(1, HALF)

    in_maps = []
    for c in range(NCORES):
        b, hg = c // 2, c % 2
        sl = slice(hg * O, (hg + 1) * O)
        in_maps.append(
            {
                "xt": np.ascontiguousarray(x[b].T.astype(np.float16)),
                "eft": np.ascontiguousarray(ef[b].T.astype(np.float16)),
                "wqt": np.ascontiguousarray(Wq[sl].T.astype(np.float16)),
                "wkt": np.ascontiguousarray(Wk[sl].T.astype(np.float16)),
                "wvt": np.ascontiguousarray(Wv[sl].T.astype(np.float16)),
                "bq": f32c(bq[sl]).reshape(1, O),
                "bk": f32c(bk[sl]).reshape(1, O),
                "bv": f32c(bv[sl]).reshape(1, O),
                "lq1": lq1,
                "lq2": lq2,
                "lk1": lk1,
                "lk2": lk2,
            }
        )

    res = st["run"](st["nc"], in_maps, core_ids=list(range(NCORES)))
    _STATE["last_results"] = res

    out = np.empty((B, T, HIDDEN), dtype=np.float32)
    for c in range(NCORES):
        b, hg = c // 2, c % 2
        out[b, :, hg * O : (hg + 1) * O] = res.results[c]["out"]
    return out


# revision 3
# speedup vs baseline: 1.1159x; 1.1159x over previous
"""Trainium2 Bass kernel for DiffMultiHeadedAttention (differential attention).

Model (per reference):
    q = x @ Wq.T + bq; k = ef @ Wk.T + bk; v = ef @ Wv.T + bv
    lambda_full = exp(sum(lq1*lk1)) - exp(sum(lq2*lk2)) + 0.8
    att  = softmax(causal_mask(q_hh @ k_hh.T / sqrt(32)))   per 32 half-heads
    out_h = (att[2h] - lambda_full * att[2h+1]) @ v_h       per 16 heads
B=4, T=N=1024, H=16 heads of 64, 2H=32 half-heads of 32.

Sharding over 8 cores: core c = (batch b = c//2, head-group hg = c%2).
Each core owns one batch element and 8 full heads (16 half-heads) and
computes out[t, o] [1024, 512] (f16); the host reassembles (no transpose).

Design notes:
  - Inputs are host-packed partition-major ([128, IC, cols]) so each tensor
    is two large-line DMAs; they are split across the two HW DGE queues
    (Sync: ef/wk/wv, Scalar: x/wq) because each DMA costs ~650ns of queue
    issue time regardless of size. Biases/lambdas ride in two packed DMAs.
  - qk: per (oc, tcv, nt) the FOUR half-head matmuls (K=32) run concurrently
    in the four 32-row PE strips via tile_position=(base, 0), into a 4-bank
    PSUM tile [128, 4, 512]; one Scalar exp per tile ((N+352)/1.2ns makes
    fewer/bigger ACTIVATEs cheaper). Triangular mask multiplied post-exp on
    DVE for diagonal tiles.
  - av transposed: E tiles are the stationary operand (FWL-eligible 128-col
    f16 loads that hide behind the streams), vaug [128n, 65] ([v_h | 1])
    streams 65 cols -> PSUM chains [128t, 4, 66] per t-chunk, accumulated
    over n-tiles. t lands on partitions, so the softmax denominator (col 64)
    is a per-partition scalar: the diff-softmax combine is 6 small DVE
    tensor_scalar ops, and output DMAs one f16 [128, 128] block per
    (t-chunk, oc).
  - Projections (q/k per-oc, v per n-tile) are chopped into 4-matmul units
    and greedily interleaved between qk slots as PE filler so the PE stays
    dense while exp paces the sweep (HAM clock-gate stays at 8/8); ~4us of
    dummy matmuls bridge the preamble->DMA window.
"""

import math

import numpy as np

B, T, N, HIDDEN = 4, 1024, 1024, 1024
H, HEAD, HALF = 16, 64, 32
O = 512            # per-core hidden slice (8 heads * 64)
HPC = 8            # heads per core
LAMBDA_INIT = 0.8
SCALE = 1.0 / math.sqrt(HALF)
P = 128
IC = HIDDEN // P   # 8 contraction chunks
OC = O // P        # 4 output chunks of the projections
NT = N // P        # 8 n-tiles (keys)
NCORES = 8

_STATE = {}


def _build_nc():
    from contextlib import ExitStack

    import concourse.bacc as bacc
    import concourse.mybir as mybir
    import concourse.tile as tile
    from concourse.bass import ts

    f32 = mybir.dt.float32
    f16 = mybir.dt.float16
    AF = mybir.ActivationFunctionType
    ALU = mybir.AluOpType

    nc = bacc.Bacc("TRN2", target_bir_lowering=False, debug=False)

    # Host-packed inputs: [128, IC*cols] partition-major (see kernel()).
    xt_d = nc.dram_tensor("xt", [P, IC * T], f16, kind="ExternalInput")
    eft_d = nc.dram_tensor("eft", [P, IC * N], f16, kind="ExternalInput")
    wqt_d = nc.dram_tensor("wqt", [P, IC * O], f16, kind="ExternalInput")
    wkt_d = nc.dram_tensor("wkt", [P, IC * O], f16, kind="ExternalInput")
    wvt_d = nc.dram_tensor("wvt", [P, IC * O], f16, kind="ExternalInput")
    # misc1 = [bv (512) | lq1 | lk1 | lq2 | lk2 (4*32)]
    misc1_d = nc.dram_tensor("misc1", [1, O + 4 * HALF], f32, kind="ExternalInput")
    # misc2 = [bq_sb (4) | bk_sb (4)] per partition
    misc2_d = nc.dram_tensor("misc2", [P, 2 * OC], f32, kind="ExternalInput")
    out_d = nc.dram_tensor("out", [T, O], f16, kind="ExternalOutput")

    with tile.TileContext(nc) as tc:
        with ExitStack() as ctx:
            const = ctx.enter_context(tc.tile_pool(name="const", bufs=1))
            big = ctx.enter_context(tc.tile_pool(name="big", bufs=1))

            # ---- input staging (persistent; proj interleaves into phase B) ----
            efT = big.tile([P, IC, N], f16)
            wkT = big.tile([P, IC, O], f16)
            wvT = big.tile([P, IC, O], f16)
            xT = big.tile([P, IC, T], f16)
            wqT = big.tile([P, IC, O], f16)
            misc2 = const.tile([P, 2 * OC], f32)
            misc1 = const.tile([1, O + 4 * HALF], f32)

            # Sync HW DGE queue: k/v inputs + per-partition biases.
            nc.sync.dma_start(misc2, misc2_d[:])
            h4 = 4 * N
            nc.sync.dma_start(
                efT[:, 0:4, :], eft_d[:, 0:h4].rearrange("p (i n) -> p i n", n=N)
            )
            nc.sync.dma_start(
                efT[:, 4:8, :], eft_d[:, h4:].rearrange("p (i n) -> p i n", n=N)
            )
            w4 = 4 * O
            nc.sync.dma_start(
                wkT[:, 0:4, :], wkt_d[:, 0:w4].rearrange("p (i n) -> p i n", n=O)
            )
            nc.sync.dma_start(
                wkT[:, 4:8, :], wkt_d[:, w4:].rearrange("p (i n) -> p i n", n=O)
            )
            nc.sync.dma_start(
                wvT[:, 0:4, :], wvt_d[:, 0:w4].rearrange("p (i n) -> p i n", n=O)
            )
            nc.sync.dma_start(
                wvT[:, 4:8, :], wvt_d[:, w4:].rearrange("p (i n) -> p i n", n=O)
            )
            # Scalar HW DGE queue: q inputs + bv/lambdas.
            nc.scalar.dma_start(misc1, misc1_d[:])
            nc.scalar.dma_start(
                xT[:, 0:4, :], xt_d[:, 0:h4].rearrange("p (i n) -> p i n", n=T)
            )
            nc.scalar.dma_start(
                xT[:, 4:8, :], xt_d[:, h4:].rearrange("p (i n) -> p i n", n=T)
            )
            nc.scalar.dma_start(
                wqT[:, 0:4, :], wqt_d[:, 0:w4].rearrange("p (i n) -> p i n", n=O)
            )
            nc.scalar.dma_start(
                wqT[:, 4:8, :], wqt_d[:, w4:].rearrange("p (i n) -> p i n", n=O)
            )

            bq_sb = misc2[:, 0:OC]
            bk_sb = misc2[:, OC : 2 * OC]
            bv_1 = misc1[:, 0:O]
            lam_in = misc1[:, O : O + 4 * HALF].rearrange("p (a h) -> p a h", h=HALF)

            # ---- lambda_full (tiny, computed once) ----
            lam_tmp = const.tile([1, 2, HALF], f32)
            nc.vector.tensor_mul(lam_tmp[:, 0, :], lam_in[:, 0, :], lam_in[:, 1, :])
            nc.vector.tensor_mul(lam_tmp[:, 1, :], lam_in[:, 2, :], lam_in[:, 3, :])
            lam_s = const.tile([1, 2], f32)
            nc.vector.tensor_reduce(
                lam_s, lam_tmp, axis=mybir.AxisListType.X, op=ALU.add
            )
            lam_e = const.tile([1, 2], f32)
            nc.scalar.activation(lam_e, lam_s, AF.Exp)
            # lam_neg = -(e1 - e2 + 0.8) = e2 - e1 - 0.8
            lam_neg = const.tile([1, 1], f32)
            nc.vector.tensor_sub(lam_neg, lam_e[:, 1:2], lam_e[:, 0:1])
            nc.vector.tensor_scalar_add(lam_neg, lam_neg, -LAMBDA_INIT)
            # lam4 = [1, -lam, 1, -lam] per partition: one DVE mul scales the
            # four reciprocals of a combine tile in a single op.
            lam_negb = const.tile([P, 1], f32)
            nc.gpsimd.partition_broadcast(lam_negb, lam_neg)
            lam4 = const.tile([P, 4, 1], f32)
            nc.vector.memset(lam4, 1.0)
            nc.vector.tensor_copy(lam4[:, 1, :], lam_negb)
            nc.vector.tensor_copy(lam4[:, 3, :], lam_negb)

            # 0/1 upper-triangular mask (keep t_local >= n_local), x4 so one
            # DVE mul masks all four half-heads of an oc.
            tri4 = const.tile([P, 4, P], f16)
            neg3 = const.tile([P, 1], f32)
            nc.vector.memset(neg3, -3.0)
            nc.gpsimd.memset(tri4, 1.0)
            nc.gpsimd.affine_select(
                out=tri4,
                in_=tri4,
                compare_op=ALU.is_ge,
                fill=0.0,
                base=0,
                pattern=[[0, 4], [1, P]],
                channel_multiplier=-1,
            )

            bvb = const.tile([P, O], f32)
            nc.gpsimd.partition_broadcast(bvb, bv_1)

            # ---- persistent projection outputs ----
            qT = big.tile([P, OC, T], f16)           # [d-part, oc, t]
            kT = big.tile([P, OC, N], f16)           # [d-part, oc, n]
            vaug = big.tile([P, NT, HPC, HEAD + 1], f16)  # [n-part, nt, h, d|1]
            ones8 = const.tile([P, HPC], f32)
            nc.vector.memset(ones8, 1.0)
            for nt_ in range(NT):
                nc.vector.tensor_copy(
                    vaug[:, nt_, :, HEAD : HEAD + 1],
                    ones8[:, :].rearrange("p (a b) -> p a b", b=1),
                )

            # ---- PSUM pools: qk 4 banks + av 2 + proj 2 = 8 ----
            ps_qk = ctx.enter_context(tc.tile_pool(name="ps_qk", bufs=1, space="PSUM"))
            ps_av = ctx.enter_context(tc.tile_pool(name="ps_av", bufs=2, space="PSUM"))
            ps_pj = ctx.enter_context(tc.tile_pool(name="ps_pj", bufs=2, space="PSUM"))
            att = ctx.enter_context(tc.tile_pool(name="att", bufs=1))
            outs = ctx.enter_context(tc.tile_pool(name="outs", bufs=1))

            # ---- PE warm-up: dummy matmuls bridge the preamble->DMA window
            # so the HAM clock-gate reaches 8/8 before real work.
            wz = const.tile([32, 512], f16)
            nc.vector.memset(wz, 0.0)
            for i in range(10):
                wps = ps_pj.tile([P, 512], f32, tag="pj", name="warm")
                nc.tensor.matmul(
                    wps[0:1, :], wz[:, 0:1], wz[:, :], start=True, stop=True
                )

            # ---------- projection chain machinery ----------
            # Each chain (8 ic-matmuls + a DVE drain) is split into two
            # 4-matmul units so the filler granularity is ~0.85us.
            def make_kq_chain(wT, b_sb, boff, actT, dstT, oc, t2):
                cell = {}

                def unit_a():
                    psj = ps_pj.tile([P, 512], f32, tag="pj", name="psj")
                    cell["psj"] = psj
                    for ic in range(4):
                        nc.tensor.matmul(
                            psj,
                            wT[:, ic, ts(oc, P)],
                            actT[:, ic, ts(t2, 512)],
                            start=(ic == 0),
                            stop=False,
                        )

                def unit_b():
                    psj = cell["psj"]
                    for ic in range(4, IC):
                        nc.tensor.matmul(
                            psj,
                            wT[:, ic, ts(oc, P)],
                            actT[:, ic, ts(t2, 512)],
                            start=False,
                            stop=(ic == IC - 1),
                        )
                    nc.vector.tensor_scalar_add(
                        dstT[:, oc, ts(t2, 512)], psj, b_sb[:, boff + oc : boff + oc + 1]
                    )

                return [(4, unit_a), (4, unit_b)]

            def make_v_chain(nt_):
                cell = {}

                def unit_a():
                    psj = ps_pj.tile([P, 512], f32, tag="pj", name="psv")
                    cell["psj"] = psj
                    for ic in range(4):
                        nc.tensor.matmul(
                            psj,
                            efT[:, ic, ts(nt_, P)],
                            wvT[:, ic, :],
                            start=(ic == 0),
                            stop=False,
                        )

                def unit_b():
                    psj = cell["psj"]
                    for ic in range(4, IC):
                        nc.tensor.matmul(
                            psj,
                            efT[:, ic, ts(nt_, P)],
                            wvT[:, ic, :],
                            start=False,
                            stop=(ic == IC - 1),
                        )
                    nc.vector.tensor_add(
                        vaug[:, nt_, :, 0:HEAD],
                        psj[:].rearrange("p (h d) -> p h d", h=HPC),
                        bvb[:].rearrange("p (h d) -> p h d", h=HPC),
                    )

                return [(4, unit_a), (4, unit_b)]

            # Phase A (emitted now, DMA-paced): k(oc0), q(oc0), v(nt0, nt1).
            for t2 in range(2):
                for _, u in make_kq_chain(wkT, misc2, OC, efT, kT, 0, t2):
                    u()
            for t2 in range(2):
                for _, u in make_kq_chain(wqT, misc2, 0, xT, qT, 0, t2):
                    u()
            for nt_ in range(2):
                for _, u in make_v_chain(nt_):
                    u()

            # Filler queue for phase B (in dependency-priority order).
            fillers = []
            for nt_ in range(2, NT):
                fillers.extend(make_v_chain(nt_))
            for oc in range(1, OC):
                for t2 in range(2):
                    fillers.extend(make_kq_chain(wkT, misc2, OC, efT, kT, oc, t2))
                for t2 in range(2):
                    fillers.extend(make_kq_chain(wqT, misc2, 0, xT, qT, oc, t2))
            fq = list(reversed(fillers))  # pop() from the front

            # ---------- phase B: attention ----------
            for oc in range(OC):
                Es = {}
                for tcv in (1, 0):
                    nis = range(NT) if tcv == 1 else range(4)
                    for nt_ in nis:
                        t0 = nt_ * P
                        cs = max(t0, 512 * tcv)
                        w = 512 * (tcv + 1) - cs
                        # qk: four half-heads concurrently in the four
                        # 32-row PE strips, four PSUM banks.
                        qkps = ps_qk.tile([P, 4, 512], f32, tag="qk", name="qkps")
                        for j in range(2):
                            for s in range(2):
                                base = 64 * j + 32 * s
                                nc.tensor.matmul(
                                    qkps[:, 2 * j + s, :w],
                                    kT[base : base + 32, oc, ts(nt_, P)],
                                    qT[base : base + 32, oc, cs : cs + w],
                                    start=True,
                                    stop=True,
                                    tile_position=(base, 0),
                                )
                        E = att.tile([P, 4, 512], f16, tag="E", bufs=14, name="E")
                        # bias shifts all exps by e^-3 (cancels in P/s),
                        # keeping E inside fp16 range
                        nc.scalar.activation(
                            E[:, :, :w],
                            qkps[:, :, :w],
                            AF.Exp,
                            bias=neg3[:, 0:1],
                            scale=SCALE,
                        )
                        if cs == t0:
                            # diagonal block: keep t_local >= n_local
                            nc.vector.tensor_mul(E[:, :, 0:P], E[:, :, 0:P], tri4)
                        Es[(tcv, nt_)] = E

                        # av chains for t-chunk tc == nt (global), then the
                        # per-partition diff-softmax combine + output DMA.
                        tc_ = nt_
                        av_units = 0.0
                        if tcv == 0 or nt_ >= 4:
                            avp = ps_av.tile([P, 4, 66], f32, tag="av", name="avp")
                            for j in range(2):
                                h = 2 * oc + j
                                for s in range(2):
                                    for ntp in range(tc_ + 1):
                                        Ew = Es[(tcv, ntp)]
                                        csp = max(ntp * P, 512 * tcv)
                                        c0 = tc_ * P - csp
                                        nc.tensor.matmul(
                                            avp[:, 2 * j + s, 0:65],
                                            Ew[:, 2 * j + s, c0 : c0 + P],
                                            vaug[:, ntp, h, :],
                                            start=(ntp == 0),
                                            stop=(ntp == tc_),
                                        )
                            av_units = 0.6 * 4 * (tc_ + 1) * 65 / 512
                            R = outs.tile([P, 4, 1], f32, tag="R", bufs=4, name="R")
                            nc.vector.reciprocal_approx_fast(
                                out=R, in_=avp[:, :, 64:65]
                            )
                            nc.vector.tensor_mul(R, R, lam4)
                            osb = outs.tile(
                                [P, 2, HEAD], f16, tag="o", bufs=4, name="osb"
                            )
                            for j in range(2):
                                nc.vector.tensor_scalar_mul(
                                    osb[:, j, :],
                                    avp[:, 2 * j + 1, 0:HEAD],
                                    R[:, 2 * j + 1, 0:1],
                                )
                                nc.vector.scalar_tensor_tensor(
                                    out=osb[:, j, :],
                                    in0=avp[:, 2 * j, 0:HEAD],
                                    scalar=R[:, 2 * j, 0:1],
                                    in1=osb[:, j, :],
                                    op0=ALU.mult,
                                    op1=ALU.add,
                                )
                            nc.sync.dma_start(
                                out_d[ts(tc_, P), ts(oc, P)],
                                osb[:].rearrange("p a b -> p (a b)"),
                            )

                        # greedy filler: keep the PE fed while exp paces
                        window = (4 * w + 352) / 1.2 / 213.0
                        acc = w / 512.0 + av_units
                        min_pops = 2 if (oc == 0 and tcv == 1 and nt_ <= 5) else 0
                        pops = 0
                        while fq and (pops < min_pops or acc < window - 1.0):
                            cost, u = fq.pop()
                            u()
                            pops += 1
                            acc += cost * 0.27
            # drain any leftover projection work (shouldn't happen)
            while fq:
                _, u = fq.pop()
                u()

    nc.compile()
    return nc


def _ensure_axon_hooks():
    """concourse's trace path imports antenv.axon_hooks, which this image
    lacks; provide it (registering the real ctypes NTFF hook when available)
    so BASS_TRACE=1 degrades gracefully instead of crashing."""
    import sys
    import types

    if "antenv.axon_hooks" in sys.modules:
        return
    try:
        import antenv.axon_hooks  # noqa: F401

        return
    except ImportError:
        pass
    mod = types.ModuleType("antenv.axon_hooks")
    mod._hook = None
    mod.set_axon_ntff_profile_hook = lambda h: setattr(mod, "_hook", h)
    mod.get_axon_ntff_profile_hook = lambda: mod._hook
    sys.modules["antenv.axon_hooks"] = mod
    import os

    if os.environ.get("KERNEL_TRACE") == "1":
        try:
            from trn_agent_boot.trn_boot import _ntff_profile_via_ctypes

            mod._hook = _ntff_profile_via_ctypes("/opt/axon/libaxon_pjrt.so")
        except Exception:
            pass


def _get_state():
    if "nc" not in _STATE:
        from concourse.bass_utils import run_bass_kernel_spmd

        _ensure_axon_hooks()
        _STATE["nc"] = _build_nc()
        _STATE["run"] = run_bass_kernel_spmd
    return _STATE


def _pack_pm(a):
    """[IC*128, cols] -> [128, IC*cols] partition-major fp16."""
    ic = a.shape[0] // P
    return np.ascontiguousarray(
        a.reshape(ic, P, a.shape[1]).transpose(1, 0, 2).reshape(P, -1).astype(np.float16)
    )


def kernel(**inputs):
    st = _get_state()

    def f32c(a):
        return np.ascontiguousarray(np.asarray(a, dtype=np.float32))

    x = np.asarray(inputs["x"], dtype=np.float32)
    ef = np.asarray(inputs["encoder_feature"], dtype=np.float32)
    Wq, bq = np.asarray(inputs["Wq"], np.float32), np.asarray(inputs["bq"], np.float32)
    Wk, bk = np.asarray(inputs["Wk"], np.float32), np.asarray(inputs["bk"], np.float32)
    Wv, bv = np.asarray(inputs["Wv"], np.float32), np.asarray(inputs["bv"], np.float32)
    lam_flat = np.concatenate(
        [
            f32c(inputs["lambda_q1"]).ravel(),
            f32c(inputs["lambda_k1"]).ravel(),
            f32c(inputs["lambda_q2"]).ravel(),
            f32c(inputs["lambda_k2"]).ravel(),
        ]
    )

    in_maps = []
    for c in range(NCORES):
        b, hg = c // 2, c % 2
        sl = slice(hg * O, (hg + 1) * O)
        misc1 = np.concatenate([bv[sl], lam_flat]).reshape(1, -1).astype(np.float32)
        misc2 = np.stack(
            [bq[sl].reshape(OC, P).T, bk[sl].reshape(OC, P).T], axis=1
        )  # [P, 2, OC]
        misc2 = np.ascontiguousarray(
            misc2.reshape(P, 2 * OC).astype(np.float32)
        )
        in_maps.append(
            {
                "xt": _pack_pm(x[b].T),
                "eft": _pack_pm(ef[b].T),
                "wqt": _pack_pm(Wq[sl].T),
                "wkt": _pack_pm(Wk[sl].T),
                "wvt": _pack_pm(Wv[sl].T),
                "misc1": np.ascontiguousarray(misc1),
                "misc2": misc2,
            }
        )

    res = st["run"](st["nc"], in_maps, core_ids=list(range(NCORES)))
    _STATE["last_results"] = res

    out = np.empty((B, T, HIDDEN), dtype=np.float32)
    for c in range(NCORES):
        b, hg = c // 2, c % 2
        out[b, :, hg * O : (hg + 1) * O] = res.results[c]["out"].astype(np.float32)
    return out


# revision 12
# speedup vs baseline: 1.2693x; 1.1375x over previous
"""Trainium2 Bass kernel for DiffMultiHeadedAttention (differential attention).

Model (per reference):
    q = x @ Wq.T + bq; k = ef @ Wk.T + bk; v = ef @ Wv.T + bv
    lambda_full = exp(sum(lq1*lk1)) - exp(sum(lq2*lk2)) + 0.8
    att  = softmax(causal_mask(q_hh @ k_hh.T / sqrt(32)))   per 32 half-heads
    out_h = (att[2h] - lambda_full * att[2h+1]) @ v_h       per 16 heads
B=4, T=N=1024, H=16 heads of 64, 2H=32 half-heads of 32.

Sharding over 8 cores: core c = (batch b = c//2, head-group hg = c%2).
Each core owns one batch element and 8 full heads (16 half-heads) and
computes out[t, o] [1024, 512] (f16); the host reassembles (no transpose).

Design notes:
  - Inputs are host-packed partition-major ([128, IC, cols]) so each tensor
    is two large-line DMAs; they are split across the two HW DGE queues
    (Sync: ef/wk/wv, Scalar: x/wq) because each DMA costs ~650ns of queue
    issue time regardless of size. Biases/lambdas ride in two packed DMAs.
  - qk: per (oc, tcv, nt) the FOUR half-head matmuls (K=32) run concurrently
    in the four 32-row PE strips via tile_position=(base, 0), into a 4-bank
    PSUM tile [128, 4, 512]; one Scalar exp per tile ((N+352)/1.2ns makes
    fewer/bigger ACTIVATEs cheaper). Triangular mask multiplied post-exp on
    DVE for diagonal tiles.
  - av transposed: E tiles are the stationary operand (FWL-eligible 128-col
    f16 loads that hide behind the streams), vaug [128n, 65] ([v_h | 1])
    streams 65 cols -> PSUM chains [128t, 4, 66] per t-chunk, accumulated
    over n-tiles. t lands on partitions, so the softmax denominator (col 64)
    is a per-partition scalar: the diff-softmax combine is 6 small DVE
    tensor_scalar ops, and output DMAs one f16 [128, 128] block per
    (t-chunk, oc).
  - Projections (q/k per-oc, v per n-tile) are chopped into 4-matmul units
    and greedily interleaved between qk slots as PE filler so the PE stays
    dense while exp paces the sweep (HAM clock-gate stays at 8/8); ~4us of
    dummy matmuls bridge the preamble->DMA window.
"""

import math

import numpy as np

B, T, N, HIDDEN = 4, 1024, 1024, 1024
H, HEAD, HALF = 16, 64, 32
O = 512            # per-core hidden slice (8 heads * 64)
HPC = 8            # heads per core
LAMBDA_INIT = 0.8
SCALE = 1.0 / math.sqrt(HALF)
P = 128
IC = HIDDEN // P   # 8 contraction chunks
OC = O // P        # 4 output chunks of the projections
NT = N // P        # 8 n-tiles (keys)
NCORES = 8

_STATE = {}


def _build_nc():
    from contextlib import ExitStack

    import concourse.bacc as bacc
    import concourse.mybir as mybir
    import concourse.tile as tile
    from concourse.bass import ts

    f32 = mybir.dt.float32
    f16 = mybir.dt.float16
    AF = mybir.ActivationFunctionType
    ALU = mybir.AluOpType

    nc = bacc.Bacc("TRN2", target_bir_lowering=False, debug=False)

    # Host-packed inputs: [128, IC*cols] partition-major (see kernel()).
    xt_d = nc.dram_tensor("xt", [P, IC * T], f16, kind="ExternalInput")
    eft_d = nc.dram_tensor("eft", [P, IC * N], f16, kind="ExternalInput")
    wqt_d = nc.dram_tensor("wqt", [P, IC * O], f16, kind="ExternalInput")
    wkt_d = nc.dram_tensor("wkt", [P, IC * O], f16, kind="ExternalInput")
    wvt_d = nc.dram_tensor("wvt", [P, IC * O], f16, kind="ExternalInput")
    # misc1 = [bv (512) | lq1 | lk1 | lq2 | lk2 (4*32)]
    misc1_d = nc.dram_tensor("misc1", [1, O + 4 * HALF], f32, kind="ExternalInput")
    # misc2 = [bq_sb (4) | bk_sb (4)] per partition
    misc2_d = nc.dram_tensor("misc2", [P, 2 * OC], f32, kind="ExternalInput")
    out_d = nc.dram_tensor("out", [T, O], f16, kind="ExternalOutput")

    with tile.TileContext(nc) as tc:
        with ExitStack() as ctx:
            const = ctx.enter_context(tc.tile_pool(name="const", bufs=1))
            big = ctx.enter_context(tc.tile_pool(name="big", bufs=1))

            # ---- input staging (persistent; proj interleaves into phase B) ----
            efT = big.tile([P, IC, N], f16)
            wkT = big.tile([P, IC, O], f16)
            wvT = big.tile([P, IC, O], f16)
            xT = big.tile([P, IC, T], f16)
            wqT = big.tile([P, IC, O], f16)
            misc2 = const.tile([P, 2 * OC], f32)
            misc1 = const.tile([1, O + 4 * HALF], f32)

            # Sync HW DGE queue: k weights first (small), then ef, then v
            # weights; transfers run in order per queue and the two queues
            # share HBM bandwidth, so gating inputs go first.
            h4 = 4 * N
            w4 = 4 * O
            nc.sync.dma_start(misc2, misc2_d[:])
            nc.sync.dma_start(
                wkT[:, 0:4, :], wkt_d[:, 0:w4].rearrange("p (i n) -> p i n", n=O)
            )
            nc.sync.dma_start(
                wkT[:, 4:8, :], wkt_d[:, w4:].rearrange("p (i n) -> p i n", n=O)
            )
            nc.sync.dma_start(
                efT[:, 0:4, :], eft_d[:, 0:h4].rearrange("p (i n) -> p i n", n=N)
            )
            nc.sync.dma_start(
                efT[:, 4:8, :], eft_d[:, h4:].rearrange("p (i n) -> p i n", n=N)
            )
            nc.sync.dma_start(
                wvT[:, 0:4, :], wvt_d[:, 0:w4].rearrange("p (i n) -> p i n", n=O)
            )
            nc.sync.dma_start(
                wvT[:, 4:8, :], wvt_d[:, w4:].rearrange("p (i n) -> p i n", n=O)
            )
            # Scalar HW DGE queue: q weights, then x + bv/lambdas.
            nc.scalar.dma_start(misc1, misc1_d[:])
            nc.scalar.dma_start(
                wqT[:, 0:4, :], wqt_d[:, 0:w4].rearrange("p (i n) -> p i n", n=O)
            )
            nc.scalar.dma_start(
                wqT[:, 4:8, :], wqt_d[:, w4:].rearrange("p (i n) -> p i n", n=O)
            )
            nc.scalar.dma_start(
                xT[:, 0:4, :], xt_d[:, 0:h4].rearrange("p (i n) -> p i n", n=T)
            )
            nc.scalar.dma_start(
                xT[:, 4:8, :], xt_d[:, h4:].rearrange("p (i n) -> p i n", n=T)
            )

            bq_sb = misc2[:, 0:OC]
            bk_sb = misc2[:, OC : 2 * OC]
            bv_1 = misc1[:, 0:O]
            lam_in = misc1[:, O : O + 4 * HALF].rearrange("p (a h) -> p a h", h=HALF)

            # ---- lambda_full (tiny, computed once) ----
            lam_tmp = const.tile([1, 2, HALF], f32)
            nc.vector.tensor_mul(lam_tmp[:, 0, :], lam_in[:, 0, :], lam_in[:, 1, :])
            nc.vector.tensor_mul(lam_tmp[:, 1, :], lam_in[:, 2, :], lam_in[:, 3, :])
            lam_s = const.tile([1, 2], f32)
            nc.vector.tensor_reduce(
                lam_s, lam_tmp, axis=mybir.AxisListType.X, op=ALU.add
            )
            lam_e = const.tile([1, 2], f32)
            nc.scalar.activation(lam_e, lam_s, AF.Exp)
            # lam_neg = -(e1 - e2 + 0.8) = e2 - e1 - 0.8
            lam_neg = const.tile([1, 1], f32)
            nc.vector.tensor_sub(lam_neg, lam_e[:, 1:2], lam_e[:, 0:1])
            nc.vector.tensor_scalar_add(lam_neg, lam_neg, -LAMBDA_INIT)
            # lam4 = [1, -lam, 1, -lam] per partition: one DVE mul scales the
            # four reciprocals of a combine tile in a single op.
            lam_negb = const.tile([P, 1], f32)
            nc.gpsimd.partition_broadcast(lam_negb, lam_neg)
            lam4 = const.tile([P, 4, 1], f32)
            nc.vector.memset(lam4, 1.0)
            nc.vector.tensor_copy(lam4[:, 1, :], lam_negb)
            nc.vector.tensor_copy(lam4[:, 3, :], lam_negb)

            # 0/1 upper-triangular mask (keep t_local >= n_local), x4 so one
            # DVE mul masks all four half-heads of an oc.
            tri4 = const.tile([P, 4, P], f16)
            neg3 = const.tile([P, 1], f32)
            nc.vector.memset(neg3, -3.0)
            nc.gpsimd.memset(tri4, 1.0)
            nc.gpsimd.affine_select(
                out=tri4,
                in_=tri4,
                compare_op=ALU.is_ge,
                fill=0.0,
                base=0,
                pattern=[[0, 4], [1, P]],
                channel_multiplier=-1,
            )

            bvb = const.tile([P, O], f32)
            nc.gpsimd.partition_broadcast(bvb, bv_1)

            # ---- persistent projection outputs ----
            qT = big.tile([P, OC, T], f16)           # [d-part, oc, t]
            kT = big.tile([P, OC, N], f16)           # [d-part, oc, n]
            vaug = big.tile([P, NT, HPC, HEAD + 1], f16)  # [n-part, nt, h, d|1]
            ones8 = const.tile([P, HPC], f32)
            nc.vector.memset(ones8, 1.0)
            for nt_ in range(NT):
                nc.vector.tensor_copy(
                    vaug[:, nt_, :, HEAD : HEAD + 1],
                    ones8[:, :].rearrange("p (a b) -> p a b", b=1),
                )

            # ---- PSUM pools: qk 4 banks + av 2 + proj 2 = 8 ----
            ps_qk = ctx.enter_context(tc.tile_pool(name="ps_qk", bufs=1, space="PSUM"))
            ps_av = ctx.enter_context(tc.tile_pool(name="ps_av", bufs=2, space="PSUM"))
            ps_pj = ctx.enter_context(tc.tile_pool(name="ps_pj", bufs=2, space="PSUM"))
            att = ctx.enter_context(tc.tile_pool(name="att", bufs=1))
            outs = ctx.enter_context(tc.tile_pool(name="outs", bufs=1))

            # ---- PE warm-up: dummy matmuls bridge the preamble->DMA window
            # so the HAM clock-gate reaches 8/8 before real work.
            wz = const.tile([32, 512], f16)
            nc.vector.memset(wz, 0.0)
            for i in range(14):
                wps = ps_pj.tile([P, 512], f32, tag="pj", name="warm")
                nc.tensor.matmul(
                    wps[0:1, :], wz[:, 0:1], wz[:, :], start=True, stop=True
                )

            # ---------- projection chain machinery ----------
            # Each chain (8 ic-matmuls + a DVE drain) is split into two
            # 4-matmul units so the filler granularity is ~0.85us.
            def make_kq_chain(wT, b_sb, boff, actT, dstT, oc, t2):
                cell = {}

                def unit_a():
                    psj = ps_pj.tile([P, 512], f32, tag="pj", name="psj")
                    cell["psj"] = psj
                    for ic in range(4):
                        nc.tensor.matmul(
                            psj,
                            wT[:, ic, ts(oc, P)],
                            actT[:, ic, ts(t2, 512)],
                            start=(ic == 0),
                            stop=False,
                        )

                def unit_b():
                    psj = cell["psj"]
                    for ic in range(4, IC):
                        nc.tensor.matmul(
                            psj,
                            wT[:, ic, ts(oc, P)],
                            actT[:, ic, ts(t2, 512)],
                            start=False,
                            stop=(ic == IC - 1),
                        )
                    nc.vector.tensor_scalar_add(
                        dstT[:, oc, ts(t2, 512)], psj, b_sb[:, boff + oc : boff + oc + 1]
                    )

                return [(4, unit_a), (4, unit_b)]

            def make_v_chain(nt_):
                cell = {}

                def unit_a():
                    psj = ps_pj.tile([P, 512], f32, tag="pj", name="psv")
                    cell["psj"] = psj
                    for ic in range(4):
                        nc.tensor.matmul(
                            psj,
                            efT[:, ic, ts(nt_, P)],
                            wvT[:, ic, :],
                            start=(ic == 0),
                            stop=False,
                        )

                def unit_b():
                    psj = cell["psj"]
                    for ic in range(4, IC):
                        nc.tensor.matmul(
                            psj,
                            efT[:, ic, ts(nt_, P)],
                            wvT[:, ic, :],
                            start=False,
                            stop=(ic == IC - 1),
                        )
                    nc.vector.tensor_add(
                        vaug[:, nt_, :, 0:HEAD],
                        psj[:].rearrange("p (h d) -> p h d", h=HPC),
                        bvb[:].rearrange("p (h d) -> p h d", h=HPC),
                    )

                return [(4, unit_a), (4, unit_b)]

            # Phase A (emitted now, DMA-paced): k(oc0), q(oc0).
            for t2 in range(2):
                for _, u in make_kq_chain(wkT, misc2, OC, efT, kT, 0, t2):
                    u()
            for t2 in range(2):
                for _, u in make_kq_chain(wqT, misc2, 0, xT, qT, 0, t2):
                    u()

            # Filler units for phase B, keyed so consumers can force-emit
            # their producers (program order defines Tile dependencies: a
            # read emitted before its producer's write sees stale data).
            units = {}
            order = []

            def add_units(key, us):
                units[key] = [u for _, u in us]
                order.append(key)

            for nt_ in range(NT):
                add_units(("v", nt_), make_v_chain(nt_))
            for oc_ in range(1, OC):
                add_units(
                    ("k", oc_, 0), make_kq_chain(wkT, misc2, OC, efT, kT, oc_, 0)
                )
                add_units(
                    ("q", oc_, 1), make_kq_chain(wqT, misc2, 0, xT, qT, oc_, 1)
                )
                add_units(
                    ("k", oc_, 1), make_kq_chain(wkT, misc2, OC, efT, kT, oc_, 1)
                )
                add_units(
                    ("q", oc_, 0), make_kq_chain(wqT, misc2, 0, xT, qT, oc_, 0)
                )

            def emit_key(key):
                for u in units.pop(key, []):
                    u()

            def pop_one():
                while order and order[0] not in units:
                    order.pop(0)
                if not order:
                    return False
                key = order[0]
                us = units[key]
                us.pop(0)()
                if not us:
                    del units[key]
                    order.pop(0)
                return True

            def supply():
                return sum(len(v) for v in units.values())

            n_slots = OC * 12
            slots_left = n_slots
            allow = 0.0

            # ---------- phase B: attention ----------
            for oc in range(OC):
                Es = {}
                for tcv in (1, 0):
                    nis = range(NT) if tcv == 1 else range(4)
                    for nt_ in nis:
                        t0 = nt_ * P
                        cs = max(t0, 512 * tcv)
                        w = 512 * (tcv + 1) - cs
                        # correctness gates: emit producers of this qk's
                        # kT/qT slices if the ration hasn't already
                        if oc > 0:
                            emit_key(("k", oc, nt_ // 4))
                            emit_key(("q", oc, tcv))
                        # qk: four half-heads concurrently in the four
                        # 32-row PE strips, four PSUM banks.
                        qkps = ps_qk.tile([P, 4, 512], f32, tag="qk", name="qkps")
                        for j in range(2):
                            for s in range(2):
                                base = 64 * j + 32 * s
                                nc.tensor.matmul(
                                    qkps[:, 2 * j + s, :w],
                                    kT[base : base + 32, oc, ts(nt_, P)],
                                    qT[base : base + 32, oc, cs : cs + w],
                                    start=True,
                                    stop=True,
                                    tile_position=(base, 0),
                                )
                        E = att.tile([P, 4, 512], f16, tag="E", bufs=14, name="E")
                        # bias shifts all exps by e^-3 (cancels in P/s),
                        # keeping E inside fp16 range
                        nc.scalar.activation(
                            E[:, :, :w],
                            qkps[:, :, :w],
                            AF.Exp,
                            bias=neg3[:, 0:1],
                            scale=SCALE,
                        )
                        if cs == t0:
                            # diagonal block: keep t_local >= n_local
                            nc.vector.tensor_mul(E[:, :, 0:P], E[:, :, 0:P], tri4)
                        Es[(tcv, nt_)] = E

                        # av chains for t-chunk tc == nt (global), then the
                        # per-partition diff-softmax combine + output DMA.
                        tc_ = nt_
                        if tcv == 0 or nt_ >= 4:
                            for ntp in range(tc_ + 1):
                                emit_key(("v", ntp))
                            avp = ps_av.tile([P, 4, 66], f32, tag="av", name="avp")
                            for j in range(2):
                                h = 2 * oc + j
                                for s in range(2):
                                    for ntp in range(tc_ + 1):
                                        Ew = Es[(tcv, ntp)]
                                        csp = max(ntp * P, 512 * tcv)
                                        c0 = tc_ * P - csp
                                        nc.tensor.matmul(
                                            avp[:, 2 * j + s, 0:65],
                                            Ew[:, 2 * j + s, c0 : c0 + P],
                                            vaug[:, ntp, h, :],
                                            start=(ntp == 0),
                                            stop=(ntp == tc_),
                                        )
                            av_units = 0.6 * 4 * (tc_ + 1) * 65 / 512
                            R = outs.tile([P, 4, 1], f32, tag="R", bufs=4, name="R")
                            nc.vector.reciprocal_approx_fast(
                                out=R, in_=avp[:, :, 64:65]
                            )
                            nc.vector.tensor_mul(R, R, lam4)
                            osb = outs.tile(
                                [P, 2, HEAD], f16, tag="o", bufs=4, name="osb"
                            )
                            for j in range(2):
                                nc.vector.tensor_scalar_mul(
                                    osb[:, j, :],
                                    avp[:, 2 * j + 1, 0:HEAD],
                                    R[:, 2 * j + 1, 0:1],
                                )
                                nc.vector.scalar_tensor_tensor(
                                    out=osb[:, j, :],
                                    in0=avp[:, 2 * j, 0:HEAD],
                                    scalar=R[:, 2 * j, 0:1],
                                    in1=osb[:, j, :],
                                    op0=ALU.mult,
                                    op1=ALU.add,
                                )
                            nc.sync.dma_start(
                                out_d[ts(tc_, P), ts(oc, P)],
                                osb[:].rearrange("p a b -> p (a b)"),
                            )

                        # Rationed filler: spread projection units evenly
                        # across all slots so the PE never has gap bursts
                        # (HAM stays 8/8). The first six oc0 slots force two
                        # units each so v(nt0..4) is ready for the first av.
                        min_pops = 2 if (oc == 0 and tcv == 1 and nt_ <= 5) else 0
                        allow += supply() / max(slots_left, 1.0)
                        slots_left -= 1
                        pops = 0
                        while (pops < min_pops or allow >= 1.0) and pop_one():
                            pops += 1
                            allow -= 1.0
            # drain any leftover projection work
            while pop_one():
                pass

    nc.compile()
    return nc


def _ensure_axon_hooks():
    """concourse's trace path imports antenv.axon_hooks, which this image
    lacks; provide it (registering the real ctypes NTFF hook when available)
    so BASS_TRACE=1 degrades gracefully instead of crashing."""
    import sys
    import types

    if "antenv.axon_hooks" in sys.modules:
        return
    try:
        import antenv.axon_hooks  # noqa: F401

        return
    except ImportError:
        pass
    mod = types.ModuleType("antenv.axon_hooks")
    mod._hook = None
    mod.set_axon_ntff_profile_hook = lambda h: setattr(mod, "_hook", h)
    mod.get_axon_ntff_profile_hook = lambda: mod._hook
    sys.modules["antenv.axon_hooks"] = mod
    import os

    if os.environ.get("KERNEL_TRACE") == "1":
        try:
            from trn_agent_boot.trn_boot import _ntff_profile_via_ctypes

            mod._hook = _ntff_profile_via_ctypes("/opt/axon/libaxon_pjrt.so")
        except Exception:
            pass


def _get_state():
    if "nc" not in _STATE:
        from concourse.bass_utils import run_bass_kernel_spmd

        _ensure_axon_hooks()
        _STATE["nc"] = _build_nc()
        _STATE["run"] = run_bass_kernel_spmd
    return _STATE


def _pack_pm(a):
    """[IC*128, cols] -> [128, IC*cols] partition-major fp16."""
    ic = a.shape[0] // P
    return np.ascontiguousarray(
        a.reshape(ic, P, a.shape[1]).transpose(1, 0, 2).reshape(P, -1).astype(np.float16)
    )


def kernel(**inputs):
    st = _get_state()

    def f32c(a):
        return np.ascontiguousarray(np.asarray(a, dtype=np.float32))

    x = np.asarray(inputs["x"], dtype=np.float32)
    ef = np.asarray(inputs["encoder_feature"], dtype=np.float32)
    Wq, bq = np.asarray(inputs["Wq"], np.float32), np.asarray(inputs["bq"], np.float32)
    Wk, bk = np.asarray(inputs["Wk"], np.float32), np.asarray(inputs["bk"], np.float32)
    Wv, bv = np.asarray(inputs["Wv"], np.float32), np.asarray(inputs["bv"], np.float32)
    lam_flat = np.concatenate(
        [
            f32c(inputs["lambda_q1"]).ravel(),
            f32c(inputs["lambda_k1"]).ravel(),
            f32c(inputs["lambda_q2"]).ravel(),
            f32c(inputs["lambda_k2"]).ravel(),
        ]
    )

    in_maps = []
    for c in range(NCORES):
        b, hg = c // 2, c % 2
        sl = slice(hg * O, (hg + 1) * O)
        misc1 = np.concatenate([bv[sl], lam_flat]).reshape(1, -1).astype(np.float32)
        misc2 = np.stack(
            [bq[sl].reshape(OC, P).T, bk[sl].reshape(OC, P).T], axis=1
        )  # [P, 2, OC]
        misc2 = np.ascontiguousarray(
            misc2.reshape(P, 2 * OC).astype(np.float32)
        )
        in_maps.append(
            {
                "xt": _pack_pm(x[b].T),
                "eft": _pack_pm(ef[b].T),
                "wqt": _pack_pm(Wq[sl].T),
                "wkt": _pack_pm(Wk[sl].T),
                "wvt": _pack_pm(Wv[sl].T),
                "misc1": np.ascontiguousarray(misc1),
                "misc2": misc2,
            }
        )

    res = st["run"](st["nc"], in_maps, core_ids=list(range(NCORES)))
    _STATE["last_results"] = res

    out = np.empty((B, T, HIDDEN), dtype=np.float32)
    for c in range(NCORES):
        b, hg = c // 2, c % 2
        out[b, :, hg * O : (hg + 1) * O] = res.results[c]["out"].astype(np.float32)
    return out
